# revision 3
# baseline (speedup 1.0000x reference)
"""2-layer GCN (PyG GCNConv x2 + ReLU) on Trainium2, 8 NeuronCores.

Self-contained Bass/Tile kernel for nn_Encoder_67396626808850.

Strategy (edge/dst-partition sharding per the sharding hint):
  - Core c owns dst nodes [c*12500, (c+1)*12500) and all edges into them.
  - Normalization folded: hs = dinv * (x @ W); per layer
    out = relu(dinv * (segsum_{e->n} hs[src_e] + hs_n) + b); layer 2
    aggregates before its matmul: out2 = relu(dinv*((agg2+hs2) @ W2) + b2).
  - hs tables are computed shard-wise (node-major matmul via lhsT = xs^T
    tiles) and AllGathered across the 8 cores.
  - Edge phase uses GPSIMD dma_gather (512B rows, int16 idx, 4 table
    chunks) + dma_scatter_add into SBUF-resident per-core aggregation
    tables (parity mode, tokens_per_rank=128).
  - Edges are organized host-side into ROUNDS: within one scatter call all
    dst rows are unique (the SDMA read-modify-write pipeline is not atomic
    for nearby duplicate rows); successive calls are serialized by the
    Tile framework's WAW dependency on the aggregation tiles.
"""

import sys

import numpy as np

sys.path.insert(0, "/opt/trn_rl_repo")

import concourse.bacc as bacc  # noqa: E402
import concourse.mybir as mybir  # noqa: E402
import concourse.tile as tile  # noqa: E402
from concourse.bass_utils import run_bass_kernel_spmd  # noqa: E402
from concourse.masks import make_identity  # noqa: E402

F32 = mybir.dt.float32
I16 = mybir.dt.int16

# problem constants (hardcoded per spec)
N_NODES = 100000
N_EDGES = 1600000
IN_CH = 256
HID = 128
OUT_CH = 64
NC = 8
BR = 48  # msg tile rows per call; call size cap = BR*128 idxs

PART = N_NODES // NC  # 12500
NSLOT = (PART + 127) // 128  # 98
PPAD = NSLOT * 128  # 12544
GROUPS = NSLOT // 2 + 1  # 50
CPC = max(1, min(NC, 32768 // PPAD))  # cores per gather chunk (2)
NCHUNK = NC // CPC  # 4
CHROWS = CPC * PPAD  # 25088


def _table_row(node):
    c, l = node // PART, node % PART
    p, s = l % 128, l // 128
    return c * PPAD + p * NSLOT + s


def _cumcount(key, n):
    order = np.lexsort((key,))
    k_sorted = key[order]
    newgrp = np.ones(n, dtype=bool)
    newgrp[1:] = k_sorted[1:] != k_sorted[:-1]
    idx_of_start = np.flatnonzero(newgrp)
    grp_id = np.cumsum(newgrp) - 1
    rank_sorted = np.arange(n) - idx_of_start[grp_id]
    rank = np.empty(n, dtype=np.int64)
    rank[order] = rank_sorted
    return rank


def _wrap16(stream):
    a = stream.reshape(-1, 16).T.copy()  # [16, L/16]
    return np.tile(a, (8, 1))  # [128, L/16]


def _preprocess(x, edge_index, W1, b1, W2, b2):
    x = np.asarray(x, dtype=np.float32)
    W1 = np.asarray(W1, dtype=np.float32)
    W2 = np.asarray(W2, dtype=np.float32)
    b1 = np.asarray(b1, dtype=np.float32)
    b2 = np.asarray(b2, dtype=np.float32)
    src = np.asarray(edge_index[0], dtype=np.int64)
    dst = np.asarray(edge_index[1], dtype=np.int64)
    e = src.shape[0]

    deg = np.bincount(dst, minlength=N_NODES).astype(np.float32) + np.float32(1.0)
    dinv = (np.float32(1.0) / np.sqrt(deg)).astype(np.float32)
    xs = x * dinv[:, None]

    core_of = dst // PART
    dst_local = dst - core_of * PART
    srow = _table_row(src)
    chunk = srow // CHROWS
    sloc = srow % CHROWS

    key_node = (core_of * NCHUNK + chunk) * PART + dst_local
    rnd = _cumcount(key_node, e)
    maxr = int(rnd.max()) + 1
    key_round = (core_of * NCHUNK + chunk) * maxr + rnd
    pos_in_rnd = _cumcount(key_round, e)

    cnt = np.bincount(key_round, minlength=NC * NCHUNK * maxr).reshape(
        NC, NCHUNK, maxr
    )
    n_r = cnt.max(axis=0)
    n_r = ((n_r + 127) // 128) * 128

    bmax = BR * 128
    calls = []
    round_ofs = np.zeros((NCHUNK, maxr), dtype=np.int64)
    col_cursor = 0
    for k in range(NCHUNK):
        ck = []
        for r in range(maxr):
            sz = int(n_r[k, r])
            if sz == 0:
                continue
            round_ofs[k, r] = col_cursor * 16
            off = 0
            while off < sz:
                s = min(bmax, sz - off)
                ck.append((col_cursor, s))
                col_cursor += s // 16
                off += s
        calls.append(ck)
    totc = col_cursor

    trash_base = NSLOT * 128
    gpos = round_ofs[chunk, rnd] + pos_in_rnd
    L = totc * 16
    src_idx = np.zeros((NC, 128, totc), dtype=np.int16)
    dst_idx = np.zeros((NC, 128, totc), dtype=np.int16)
    for c in range(NC):
        sstream = np.zeros(L, dtype=np.int16)
        dstream = (trash_base + (np.arange(L) % 128)).astype(np.int16)
        m = core_of == c
        sstream[gpos[m]] = sloc[m].astype(np.int16)
        dstream[gpos[m]] = dst_local[m].astype(np.int16)
        src_idx[c] = _wrap16(sstream)
        dst_idx[c] = _wrap16(dstream)

    in_maps = []
    for c in range(NC):
        xs_c = xs[c * PART : (c + 1) * PART]
        xsT = np.zeros((IN_CH, PPAD), dtype=np.float32)
        xsT[:, :PART] = xs_c.T
        dlp = np.zeros(PPAD, dtype=np.float32)
        dlp[:PART] = dinv[c * PART : (c + 1) * PART]
        dg = dlp.reshape(NSLOT, 128).T.copy()  # dg[p, s] = dinv[s*128+p]
        in_maps.append(
            {
                "xsT": np.ascontiguousarray(xsT),
                "w1": W1,
                "w2": W2,
                "b1r": np.tile(b1[None, :], (128, 1)).astype(np.float32),
                "b2r": np.tile(b2[None, :], (128, 1)).astype(np.float32),
                "dinvg": dg,
                "srcidx": src_idx[c],
                "dstidx": dst_idx[c],
            }
        )
    return in_maps, calls, totc


def _build_program(calls, totc):
    KCH = IN_CH // 128

    nc = bacc.Bacc(
        "TRN2", target_bir_lowering=False, debug=False, num_devices=NC
    )
    xsT = nc.dram_tensor("xsT", [IN_CH, PPAD], F32, kind="ExternalInput")
    w1 = nc.dram_tensor("w1", [IN_CH, HID], F32, kind="ExternalInput")
    w2 = nc.dram_tensor("w2", [HID, OUT_CH], F32, kind="ExternalInput")
    b1r = nc.dram_tensor("b1r", [128, HID], F32, kind="ExternalInput")
    b2r = nc.dram_tensor("b2r", [128, OUT_CH], F32, kind="ExternalInput")
    dinvg = nc.dram_tensor("dinvg", [128, NSLOT], F32, kind="ExternalInput")
    srcidx = nc.dram_tensor("srcidx", [128, totc], I16, kind="ExternalInput")
    dstidx = nc.dram_tensor("dstidx", [128, totc], I16, kind="ExternalInput")
    outp = nc.dram_tensor("out", [PPAD, OUT_CH], F32, kind="ExternalOutput")

    hs_shard = [nc.dram_tensor(f"hs{i}_shard", [PPAD, HID], F32) for i in (1, 2)]
    hs_full = [
        nc.dram_tensor(f"hs{i}_full", [NC * PPAD, HID], F32, addr_space="Shared")
        for i in (1, 2)
    ]
    rg = [list(range(NC))]

    with tile.TileContext(nc) as tc:
        with (
            tc.tile_pool(name="const", bufs=1) as constp,
            tc.tile_pool(name="mm", bufs=3) as mmp,
            tc.tile_pool(name="ps", bufs=2, space="PSUM") as psp,
            tc.tile_pool(name="hsloc", bufs=1) as hslocp,
            tc.tile_pool(name="agg", bufs=1) as aggp,
            tc.tile_pool(name="msg", bufs=2) as msgp,
            tc.tile_pool(name="idx", bufs=4) as idxp,
            tc.tile_pool(name="post", bufs=3) as postp,
        ):
            w1t = []
            for kk in range(KCH):
                t = constp.tile([128, HID], F32, tag=f"w1_{kk}")
                nc.sync.dma_start(out=t[:], in_=w1[kk * 128 : (kk + 1) * 128, :])
                w1t.append(t)
            w2t = constp.tile([128, OUT_CH], F32)
            nc.sync.dma_start(out=w2t[:], in_=w2[:, :])
            b1t = constp.tile([128, HID], F32)
            nc.sync.dma_start(out=b1t[:], in_=b1r[:, :])
            b2t = constp.tile([128, OUT_CH], F32)
            nc.sync.dma_start(out=b2t[:], in_=b2r[:, :])
            dnt = constp.tile([128, NSLOT], F32)
            nc.sync.dma_start(out=dnt[:], in_=dinvg[:, :])
            ident = constp.tile([128, 128], F32)
            make_identity(nc, ident[:])

            hs_loc = hslocp.tile([128, NSLOT, HID], F32)
            aggA = aggp.tile([128, GROUPS, HID], F32)
            aggB = aggp.tile([128, GROUPS, HID], F32)
            out_loc = hslocp.tile([128, NSLOT, OUT_CH], F32)

            nev = NSLOT - NSLOT // 2
            nod = NSLOT // 2
            dn_ev = dnt[:, 0:NSLOT:2].to_broadcast([128, nev, HID])
            dn_od = dnt[:, 1:NSLOT:2].to_broadcast([128, nod, HID])
            b1_ev = (
                b1t[:].rearrange("p (x d) -> p x d", x=1).to_broadcast([128, nev, HID])
            )
            b1_od = (
                b1t[:].rearrange("p (x d) -> p x d", x=1).to_broadcast([128, nod, HID])
            )

            def mm_layer1():
                for s in range(NSLOT):
                    ps = psp.tile([128, HID], F32, space="PSUM", tag="mm1")
                    for kk in range(KCH):
                        xt = mmp.tile([128, 128], F32, tag="xt")
                        nc.sync.dma_start(
                            out=xt[:],
                            in_=xsT[
                                kk * 128 : (kk + 1) * 128, s * 128 : (s + 1) * 128
                            ],
                        )
                        nc.tensor.matmul(
                            ps[:],
                            lhsT=xt[:],
                            rhs=w1t[kk][:],
                            start=(kk == 0),
                            stop=(kk == KCH - 1),
                        )
                    nc.vector.tensor_copy(hs_loc[:, s, :], ps[:])

            def store_and_gather(i):
                shard_re = hs_shard[i].ap().rearrange("(p s) d -> p s d", p=128)
                nc.sync.dma_start(out=shard_re, in_=hs_loc[:])
                nc.gpsimd.collective_compute(
                    "AllGather",
                    mybir.AluOpType.bypass,
                    replica_groups=rg,
                    ins=[hs_shard[i].ap()],
                    outs=[hs_full[i].ap()],
                )

            def edge_phase(i):
                nc.vector.memset(aggA[:], 0.0)
                nc.vector.memset(aggB[:], 0.0)
                tbl = hs_full[i].ap()
                for k in range(NCHUNK):
                    tchunk = tbl[k * CHROWS : (k + 1) * CHROWS, :]
                    for co, sz in calls[k]:
                        nc16 = sz // 16
                        sit = idxp.tile([128, BR * 8], I16, tag="sidx")
                        nc.sync.dma_start(
                            out=sit[:, :nc16], in_=srcidx[:, co : co + nc16]
                        )
                        dit = idxp.tile([128, BR * 8], I16, tag="didx")
                        nc.sync.dma_start(
                            out=dit[:, :nc16], in_=dstidx[:, co : co + nc16]
                        )
                        msg = msgp.tile([128, BR, HID], F32, tag="msg")
                        nc.gpsimd.dma_gather(
                            msg[:, : sz // 128, :],
                            tchunk,
                            sit[:, :nc16],
                            sz,
                            sz,
                            HID,
                            single_packet=False,
                        )
                        nc.gpsimd.dma_scatter_add(
                            aggA[:],
                            msg[:, : sz // 128, :],
                            dit[:, :nc16],
                            sz,
                            sz,
                            HID,
                            sbuf_tokens_per_rank=128,
                            parity_reg=0,
                            out_ap_other=aggB[:],
                            single_packet=False,
                        )

            def post_layer1():
                for agg, hsv, dn, b1v, cnt in (
                    (aggA, hs_loc[:, 0:NSLOT:2, :], dn_ev, b1_ev, nev),
                    (aggB, hs_loc[:, 1:NSLOT:2, :], dn_od, b1_od, nod),
                ):
                    u = agg[:, :cnt, :]
                    nc.vector.tensor_add(u, u, hsv)
                    nc.vector.tensor_mul(u, u, dn)
                    nc.vector.tensor_add(u, u, b1v)
                    nc.vector.tensor_scalar_max(u, u, 0.0)
                    nc.vector.tensor_mul(hsv, u, dn)

            def post_layer2():
                for agg, hsv, cnt in (
                    (aggA, hs_loc[:, 0:NSLOT:2, :], nev),
                    (aggB, hs_loc[:, 1:NSLOT:2, :], nod),
                ):
                    u = agg[:, :cnt, :]
                    nc.vector.tensor_add(u, u, hsv)
                for s in range(NSLOT):
                    agg = aggA if s % 2 == 0 else aggB
                    g = s // 2
                    pst = psp.tile([128, 128], F32, space="PSUM", tag="tr")
                    nc.tensor.transpose(
                        out=pst[:], in_=agg[:, g, :], identity=ident[:]
                    )
                    ut = postp.tile([128, 128], F32, tag="ut")
                    nc.vector.tensor_copy(ut[:], pst[:])
                    zp = psp.tile([128, OUT_CH], F32, space="PSUM", tag="z")
                    nc.tensor.matmul(
                        zp[:], lhsT=ut[:], rhs=w2t[:], start=True, stop=True
                    )
                    zt = postp.tile([128, OUT_CH], F32, tag="zt")
                    nc.vector.tensor_scalar_mul(zt[:], zp[:], dnt[:, s : s + 1])
                    nc.vector.tensor_add(zt[:], zt[:], b2t[:])
                    nc.vector.tensor_scalar_max(out_loc[:, s, :], zt[:], 0.0)
                out_re = outp.ap().rearrange("(p s) d -> p s d", p=128)
                nc.sync.dma_start(out=out_re, in_=out_loc[:])

            mm_layer1()
            store_and_gather(0)
            edge_phase(0)
            post_layer1()
            store_and_gather(1)
            edge_phase(1)
            post_layer2()

    nc.compile()
    return nc


LAST_EXEC_NS = None
LAST_RB = None


def kernel(x, edge_index, W1, b1, W2, b2):
    global LAST_EXEC_NS, LAST_RB
    import os

    in_maps, calls, totc = _preprocess(x, edge_index, W1, b1, W2, b2)
    nc = _build_program(calls, totc)
    trace = bool(int(os.environ.get("GCN_TRACE", "0")))
    rb = run_bass_kernel_spmd(nc, in_maps, list(range(NC)), trace=trace)
    LAST_EXEC_NS = rb.exec_time_ns
    LAST_RB = rb
    out = np.empty((N_NODES, OUT_CH), dtype=np.float32)
    for c in range(NC):
        arr = rb.results[c]["out"].reshape(128, NSLOT, OUT_CH)
        full = arr.transpose(1, 0, 2).reshape(PPAD, OUT_CH)
        out[c * PART : (c + 1) * PART] = full[:PART]
    return out


# revision 22
# speedup vs baseline: 1.2491x; 1.2491x over previous
"""2-layer GCN (PyG GCNConv x2 + ReLU) on Trainium2, 8 NeuronCores.

Self-contained Bass/Tile kernel for nn_Encoder_67396626808850.

Strategy (edge/dst-partition sharding per the sharding hint):
  - Core c owns dst nodes [c*12500, (c+1)*12500) and all edges into them.
  - Normalization folded: hs = dinv * (x @ W); per layer
    out = relu(dinv * (segsum_{e->n} hs[src_e] + hs_n) + b); layer 2
    aggregates before its matmul: out2 = relu(dinv*((agg2+hs2) @ W2) + b2).
  - hs tables are computed shard-wise (node-major matmul via lhsT = xs^T
    tiles) and AllGathered across the 8 cores (4 int16-addressable chunks).
  - Edge phase: GPSIMD dma_gather pulls hs[src] rows (512B descriptors)
    for edges sorted by (chunk, dst-window); segment reduction runs on the
    TensorEngine: per dst window of 128 nodes, PSUM accumulates
    SEL^T.T @ msg where SEL[e, m] = (dst_local[e] == iota_w[m]) is built
    on the VectorEngine. No scatter instruction at all (the SWDGE
    descriptor generation on the Q7 cores is the bottleneck: ~7ns/idx for
    gather, ~14ns/idx for scatter-add).
  - SPMD uniformity: per (chunk, window) edge runs are padded to the max
    count over cores (32-edge granularity); pad edges carry an
    out-of-range dst so SEL rejects them automatically.
"""

import sys

import numpy as np

sys.path.insert(0, "/opt/trn_rl_repo")

import concourse.bacc as bacc  # noqa: E402
import concourse.mybir as mybir  # noqa: E402
import concourse.tile as tile  # noqa: E402
from concourse.bass_utils import run_bass_kernel_spmd  # noqa: E402
from concourse.masks import make_identity  # noqa: E402

F32 = mybir.dt.float32
BF16 = mybir.dt.bfloat16
I16 = mybir.dt.int16
TBL = F32  # gather-table dtype

# problem constants (hardcoded per spec)
N_NODES = 100000
N_EDGES = 1600000
IN_CH = 256
HID = 128
OUT_CH = 64
NC = 8
BR = 48  # msg tile columns per gather call (call = BR*128 idxs)
GRAN = 32  # window-run padding granularity (edges)

PART = N_NODES // NC  # 12500
NSLOT = (PART + 127) // 128  # 98 dst windows per core
PPAD = NSLOT * 128  # 12544
CPC = max(1, min(NC, 32768 // PPAD))  # cores per gather chunk (2)
NCHUNK = NC // CPC  # 4
CHROWS = CPC * PPAD  # 25088
PAD_DST = NSLOT * 128 + 256  # outside every window's iota range


def _table_row(node):
    c, l = node // PART, node % PART
    p, s = l % 128, l // 128
    return c * PPAD + p * NSLOT + s


def _cumcount(key, n):
    order = np.lexsort((key,))
    k_sorted = key[order]
    newgrp = np.ones(n, dtype=bool)
    newgrp[1:] = k_sorted[1:] != k_sorted[:-1]
    idx_of_start = np.flatnonzero(newgrp)
    grp_id = np.cumsum(newgrp) - 1
    rank_sorted = np.arange(n) - idx_of_start[grp_id]
    rank = np.empty(n, dtype=np.int64)
    rank[order] = rank_sorted
    return rank


def _wrap16(stream):
    a = stream.reshape(-1, 16).T.copy()  # [16, L/16]
    return np.tile(a, (8, 1))  # [128, L/16]


def _preprocess(x, edge_index, W1, b1, W2, b2):
    x = np.asarray(x, dtype=np.float32)
    W1 = np.asarray(W1, dtype=np.float32)
    W2 = np.asarray(W2, dtype=np.float32)
    b1 = np.asarray(b1, dtype=np.float32)
    b2 = np.asarray(b2, dtype=np.float32)
    src = np.asarray(edge_index[0], dtype=np.int64)
    dst = np.asarray(edge_index[1], dtype=np.int64)
    e = src.shape[0]

    deg = np.bincount(dst, minlength=N_NODES).astype(np.float32) + np.float32(1.0)
    dinv = (np.float32(1.0) / np.sqrt(deg)).astype(np.float32)
    xs = x * dinv[:, None]

    core_of = dst // PART
    dst_local = dst - core_of * PART
    win = dst_local // 128
    srow = _table_row(src)
    chunk = srow // CHROWS
    sloc = srow % CHROWS

    # per (core, chunk, window) counts -> capacity = max over cores, %GRAN
    key = (core_of * NCHUNK + chunk) * NSLOT + win
    cnt = np.bincount(key, minlength=NC * NCHUNK * NSLOT).reshape(
        NC, NCHUNK, NSLOT
    )
    cap = cnt.max(axis=0)  # [NCHUNK, NSLOT]
    cap = ((cap + GRAN - 1) // GRAN) * GRAN

    # stream offsets: chunk-major, window-major inside chunk; chunk streams
    # padded to a multiple of 128 positions
    Lk = cap.sum(axis=1)  # stream length per chunk
    Lk_pad = ((Lk + 127) // 128) * 128
    chunk_base = np.concatenate([[0], np.cumsum(Lk_pad)])
    win_ofs = np.zeros((NCHUNK, NSLOT), dtype=np.int64)
    for k in range(NCHUNK):
        win_ofs[k] = chunk_base[k] + np.concatenate([[0], np.cumsum(cap[k])[:-1]])
    total_pos = int(chunk_base[-1])

    rank = _cumcount(key, e)  # position within the (c,k,w) run
    gpos = win_ofs[chunk, win] + rank

    src_idx = np.zeros((NC, 128, total_pos // 16), dtype=np.int16)
    dstloc_arr = np.zeros((NC, 128, total_pos // 128), dtype=np.float32)
    for c in range(NC):
        sstream = np.zeros(total_pos, dtype=np.int16)
        dstream = np.full(total_pos, PAD_DST, dtype=np.float32)
        m = core_of == c
        sstream[gpos[m]] = sloc[m].astype(np.int16)
        dstream[gpos[m]] = dst_local[m].astype(np.float32)
        src_idx[c] = _wrap16(sstream)
        # position i -> [i%128, i//128]
        dstloc_arr[c] = dstream.reshape(-1, 128).T.copy()

    # device schedule (compile-time, uniform across cores):
    # per chunk: gather calls (pos0, sz); per window: subtile list
    gcalls = []  # [chunk] -> list of (pos0, sz)
    for k in range(NCHUNK):
        ck = []
        p0, left = int(chunk_base[k]), int(Lk_pad[k])
        while left > 0:
            s = min(BR * 128, left)
            ck.append((p0, s))
            p0 += s
            left -= s
        gcalls.append(ck)

    wsched = []  # [chunk] -> list of (w, col0, ncols) in stream columns
    for k in range(NCHUNK):
        wk = []
        for w in range(NSLOT):
            if cap[k, w] == 0:
                continue
            p0 = int(win_ofs[k, w])
            p1 = p0 + int(cap[k, w]) - 1
            wk.append((w, p0 // 128, p1 // 128 - p0 // 128 + 1))
        wsched.append(wk)

    in_maps = []
    for c in range(NC):
        xs_c = xs[c * PART : (c + 1) * PART]
        xsT = np.zeros((IN_CH, PPAD), dtype=np.float32)
        xsT[:, :PART] = xs_c.T
        dlp = np.zeros(PPAD, dtype=np.float32)
        dlp[:PART] = dinv[c * PART : (c + 1) * PART]
        dg = dlp.reshape(NSLOT, 128).T.copy()  # dg[p, s] = dinv[s*128+p]
        in_maps.append(
            {
                "xsT": np.ascontiguousarray(xsT),
                "w1": W1,
                "w2": W2,
                "b1r": np.tile(b1[None, :], (128, 1)).astype(np.float32),
                "b2r": np.tile(b2[None, :], (128, 1)).astype(np.float32),
                "dinvg": dg,
                "srcidx": src_idx[c],
                "dstloc": dstloc_arr[c],
                "iotah": np.tile(
                    np.arange(128, dtype=np.float32)[None, :], (128, 1)
                ),
            }
        )
    meta = dict(gcalls=gcalls, wsched=wsched, total_pos=total_pos)
    return in_maps, meta


def _build_program(meta, repeat=1, ablate=(), nq=4):
    KCH = IN_CH // 128
    ablate = frozenset(ablate)
    gcalls, wsched, total_pos = meta["gcalls"], meta["wsched"], meta["total_pos"]

    nc = bacc.Bacc(
        "TRN2",
        target_bir_lowering=False,
        debug=False,
        num_devices=NC,
        num_swdge_queues=nq,
    )
    xsT = nc.dram_tensor("xsT", [IN_CH, PPAD], F32, kind="ExternalInput")
    w1 = nc.dram_tensor("w1", [IN_CH, HID], F32, kind="ExternalInput")
    w2 = nc.dram_tensor("w2", [HID, OUT_CH], F32, kind="ExternalInput")
    b1r = nc.dram_tensor("b1r", [128, HID], F32, kind="ExternalInput")
    b2r = nc.dram_tensor("b2r", [128, OUT_CH], F32, kind="ExternalInput")
    dinvg = nc.dram_tensor("dinvg", [128, NSLOT], F32, kind="ExternalInput")
    srcidx = nc.dram_tensor(
        "srcidx", [128, total_pos // 16], I16, kind="ExternalInput"
    )
    dstloc = nc.dram_tensor(
        "dstloc", [128, total_pos // 128], F32, kind="ExternalInput"
    )
    iotah = nc.dram_tensor("iotah", [128, 128], F32, kind="ExternalInput")
    outp = nc.dram_tensor("out", [PPAD, OUT_CH], F32, kind="ExternalOutput")

    hs_shard = [nc.dram_tensor(f"hs{i}_shard", [PPAD, HID], TBL) for i in (1, 2)]
    hs_full = [
        nc.dram_tensor(f"hs{i}_full", [NC * PPAD, HID], TBL, addr_space="Shared")
        for i in (1, 2)
    ]
    rg = [list(range(NC))]

    with tile.TileContext(nc) as tc:
        with (
            tc.tile_pool(name="const", bufs=1) as constp,
            tc.tile_pool(name="mm", bufs=3) as mmp,
            tc.tile_pool(name="ps", bufs=2, space="PSUM") as psp,
            tc.tile_pool(name="hsloc", bufs=1) as hslocp,
            tc.tile_pool(name="agg", bufs=1) as aggp,
            tc.tile_pool(name="msg", bufs=2) as msgp,
            tc.tile_pool(name="idx", bufs=4) as idxp,
            tc.tile_pool(name="sel", bufs=4) as selp,
            tc.tile_pool(name="post", bufs=3) as postp,
        ):
            w1t = []
            for kk in range(KCH):
                t = constp.tile([128, HID], F32, tag=f"w1_{kk}")
                nc.sync.dma_start(out=t[:], in_=w1[kk * 128 : (kk + 1) * 128, :])
                w1t.append(t)
            w2t = constp.tile([128, OUT_CH], F32)
            nc.sync.dma_start(out=w2t[:], in_=w2[:, :])
            b1t = constp.tile([128, HID], F32)
            nc.sync.dma_start(out=b1t[:], in_=b1r[:, :])
            b2t = constp.tile([128, OUT_CH], F32)
            nc.sync.dma_start(out=b2t[:], in_=b2r[:, :])
            dnt = constp.tile([128, NSLOT], F32)
            nc.sync.dma_start(out=dnt[:], in_=dinvg[:, :])
            ident = constp.tile([128, 128], F32)
            make_identity(nc, ident[:])
            # iota along the free dim, same on every partition (host const)
            iota = constp.tile([128, 128], F32)
            nc.sync.dma_start(out=iota[:], in_=iotah[:, :])

            hs_loc = hslocp.tile([128, NSLOT, HID], F32)
            agg = aggp.tile([128, NSLOT, HID], F32)
            out_loc = hslocp.tile([128, NSLOT, OUT_CH], F32)

            dn_all = dnt[:].to_broadcast([128, NSLOT, HID])
            b1_all = (
                b1t[:]
                .rearrange("p (x d) -> p x d", x=1)
                .to_broadcast([128, NSLOT, HID])
            )

            def mm_layer1():
                for s in range(NSLOT):
                    ps = psp.tile([128, HID], F32, space="PSUM", tag="mm1")
                    for kk in range(KCH):
                        xt = mmp.tile([128, 128], F32, tag="xt")
                        nc.sync.dma_start(
                            out=xt[:],
                            in_=xsT[
                                kk * 128 : (kk + 1) * 128, s * 128 : (s + 1) * 128
                            ],
                        )
                        nc.tensor.matmul(
                            ps[:],
                            lhsT=xt[:],
                            rhs=w1t[kk][:],
                            start=(kk == 0),
                            stop=(kk == KCH - 1),
                        )
                    nc.vector.tensor_copy(hs_loc[:, s, :], ps[:])

            def store_and_gather(i):
                shard_re = hs_shard[i].ap().rearrange("(p s) d -> p s d", p=128)
                if TBL is F32:
                    nc.sync.dma_start(out=shard_re, in_=hs_loc[:])
                else:
                    # SWDGE cast-DMA: fp32 SBUF -> bf16 DRAM table
                    nc.gpsimd.dma_start(out=shard_re, in_=hs_loc[:])
                if "ag" in ablate:
                    return
                nc.gpsimd.collective_compute(
                    "AllGather",
                    mybir.AluOpType.bypass,
                    replica_groups=rg,
                    ins=[hs_shard[i].ap()],
                    outs=[hs_full[i].ap()],
                )

            def edge_phase(i):
                nc.vector.memset(agg[:], 0.0)
                if "edges" in ablate:
                    return
                tbl = hs_full[i].ap()
                for k in range(NCHUNK):
                    tchunk = tbl[k * CHROWS : (k + 1) * CHROWS, :]
                    base = gcalls[k][0][0]
                    span = sum(sz for _, sz in gcalls[k])
                    # msg tiles for this chunk, keyed by 128-position column
                    msgs = {}
                    if "gather" not in ablate:
                        for gi, (pos0, sz) in enumerate(gcalls[k]):
                            nc16 = sz // 16
                            co = pos0 // 16
                            sit = idxp.tile([128, BR * 8], I16, tag="sidx")
                            nc.sync.dma_start(
                                out=sit[:, :nc16], in_=srcidx[:, co : co + nc16]
                            )
                            msg = msgp.tile([128, BR, HID], TBL, tag="msg")
                            nc.gpsimd.dma_gather(
                                msg[:, : sz // 128, :],
                                tchunk,
                                sit[:, :nc16],
                                sz,
                                sz,
                                HID,
                                single_packet=False,
                                queue_num=gi % nq,
                            )
                            for j in range(sz // 128):
                                msgs[pos0 // 128 + j] = (msg, j)
                    # dst-local values for the whole chunk span
                    dcol0 = base // 128
                    ncols = span // 128
                    dt_ = idxp.tile([128, 512], F32, tag="dstloc")
                    nc.sync.dma_start(
                        out=dt_[:, :ncols], in_=dstloc[:, dcol0 : dcol0 + ncols]
                    )
                    if "sel" in ablate or "gather" in ablate:
                        continue
                    for w, col0, ncols in wsched[k]:
                        iw = selp.tile([128, 128], F32, tag="iw")
                        nc.vector.tensor_scalar_add(iw[:], iota[:], float(w * 128))
                        pw = psp.tile([128, HID], F32, space="PSUM", tag="win")
                        for ci in range(ncols):
                            colj = col0 - base // 128 + ci
                            mt, mj = msgs[col0 + ci]
                            selt = selp.tile([128, 128], TBL, tag="sel")
                            nc.vector.tensor_tensor(
                                out=selt[:],
                                in0=dt_[:, colj : colj + 1].to_broadcast([128, 128]),
                                in1=iw[:],
                                op=mybir.AluOpType.is_equal,
                            )
                            nc.tensor.matmul(
                                pw[:],
                                lhsT=selt[:],
                                rhs=mt[:, mj, :],
                                start=(ci == 0),
                                stop=(ci == ncols - 1),
                            )
                        nc.vector.tensor_add(agg[:, w, :], agg[:, w, :], pw[:])

            def post_layer1():
                u = agg[:]
                nc.vector.tensor_add(u, u, hs_loc[:])
                nc.vector.tensor_mul(u, u, dn_all)
                nc.vector.tensor_add(u, u, b1_all)
                nc.vector.tensor_scalar_max(u, u, 0.0)
                nc.vector.tensor_mul(hs_loc[:], u, dn_all)

            def post_layer2():
                nc.vector.tensor_add(agg[:], agg[:], hs_loc[:])
                for s in range(NSLOT):
                    pst = psp.tile([128, 128], F32, space="PSUM", tag="tr")
                    nc.tensor.transpose(
                        out=pst[:], in_=agg[:, s, :], identity=ident[:]
                    )
                    ut = postp.tile([128, 128], F32, tag="ut")
                    nc.vector.tensor_copy(ut[:], pst[:])
                    zp = psp.tile([128, OUT_CH], F32, space="PSUM", tag="z")
                    nc.tensor.matmul(
                        zp[:], lhsT=ut[:], rhs=w2t[:], start=True, stop=True
                    )
                    zt = postp.tile([128, OUT_CH], F32, tag="zt")
                    nc.vector.tensor_scalar_mul(zt[:], zp[:], dnt[:, s : s + 1])
                    nc.vector.tensor_add(zt[:], zt[:], b2t[:])
                    nc.vector.tensor_scalar_max(out_loc[:, s, :], zt[:], 0.0)
                out_re = outp.ap().rearrange("(p s) d -> p s d", p=128)
                nc.sync.dma_start(out=out_re, in_=out_loc[:])

            for _rep in range(repeat):
                mm_layer1()
                store_and_gather(0)
                edge_phase(0)
                post_layer1()
                store_and_gather(1)
                edge_phase(1)
                post_layer2()

    nc.compile()
    return nc


LAST_EXEC_NS = None
LAST_RB = None


def kernel(x, edge_index, W1, b1, W2, b2):
    global LAST_EXEC_NS, LAST_RB

    in_maps, meta = _preprocess(x, edge_index, W1, b1, W2, b2)
    nc = _build_program(meta)
    rb = run_bass_kernel_spmd(nc, in_maps, list(range(NC)))
    LAST_EXEC_NS = rb.exec_time_ns
    LAST_RB = rb
    out = np.empty((N_NODES, OUT_CH), dtype=np.float32)
    for c in range(NC):
        arr = rb.results[c]["out"].reshape(128, NSLOT, OUT_CH)
        full = arr.transpose(1, 0, 2).reshape(PPAD, OUT_CH)
        out[c * PART : (c + 1) * PART] = full[:PART]
    return out
